# revision 1
# baseline (speedup 1.0000x reference)
"""CKA (RBF-kernel HSIC) on 8 Trainium2 NeuronCores.

Row-shards the n=4096 samples across 8 cores. Each core computes its
[512, 4096] slab of both RBF Gram matrices tile-by-tile on-chip (never
materialized in DRAM) and emits only:
  - per-row sums of Kx and Ky               (rx, ry)
  - partial sums  S_xx = sum Kx*Kx, S_yy, S_xy  over its slab
The host combines partials in float64 via
  HSIC(Ka,Kb) = S_ab - (2/n) ra.rb + (sum Ka)(sum Kb)/n^2
which is algebraically identical to sum(center(Ka)*center(Kb)).

Device design:
  - Inputs are pre-scaled by 1/sigma on the host so the Gram matmul
    directly produces G/sigma^2 and ALL matmuls (including the column
    bias rows) run in fp8-e4m3 DoubleRow mode.
  - The column bias -|x_j|^2/(2 s^2) rides as a hi/lo fp8 pair on one
    DoubleRow partition; the row bias (exact fp32) is applied by the
    ScalarE Exp activation, whose accum_out produces row sums for free.
  - k-outer matmul loop: each stationary slab serves 4 consecutive
    512-wide matmuls (weight-load amortization); PSUM chunks are
    [128, 2048] (4 banks) double-buffered; one Exp per half-row.
  - X and Y m-tiles are interleaved (x0 y0 x1 y1 ...) so the three
    Hadamard-product sums of tile m start right after y_m, keeping
    VectorE busy throughout instead of piling up in a tail.
  - All three Hadamard-product sums (S_xx, S_yy, S_xy) run as VectorE
    scalar_tensor_tensor with accum. HW sweep result: ANY extra ScalarE
    exp(2*arg) share regresses (none=94-103us < part=128 < h1=137 <
    all=150 in one window) -- ScalarE is latency-critical between PSUM
    drain and the next matmul group, so keep it exp-only.
  - Small tensors go on the ScalarE DGE queue, bulk tensors on the SP
    queue (parallel descriptor generation), X before Y; input tiles are
    double-buffered so the next rep's loads overlap compute.
"""

import numpy as np
import ml_dtypes

BF16 = ml_dtypes.bfloat16
FP8 = ml_dtypes.float8_e4m3

N = 4096          # samples
D = 768           # features
NCORES = 8
ROWS = N // NCORES        # 512 rows per core
MT = ROWS // 128          # 4 m-tiles per core
KC2 = D // 256            # 3 DoubleRow contraction chunks (256 rows each)
HALF = 2048               # ACT/psum chunk width (4 PSUM banks)
MMN = 512                 # matmul moving free dim (one PSUM bank)

# self-product sums: DVE sst covers cols [0, HALF+EXP2_START), ScalarE exp2
# covers [EXP2_START, HALF) of the second PSUM half-chunk
EXP2_START = 896

_cache = {}
LAST_RESULTS = None   # BassKernelResults of the most recent run (for test harness)


def _build(inv_sigma_sq: float, reps: int = 1, mode: str = "full",
           exp2: str = "all", deep: bool = False):
    import concourse.bacc as bacc
    import concourse.mybir as mybir
    import concourse.tile as tile

    fp32 = mybir.dt.float32
    bf16 = mybir.dt.bfloat16
    fp8 = mybir.dt.float8e4
    DR = mybir.MatmulPerfMode.DoubleRow
    Exp = mybir.ActivationFunctionType.Exp
    mult = mybir.AluOpType.mult

    nc = bacc.Bacc(None)

    # moving operands, half-major: [2, 128, KC2, 2, HALF] (x/sigma in fp8)
    xt = nc.dram_tensor("xt", [2, 128, KC2, 2, HALF], fp8, kind="ExternalInput")
    yt = nc.dram_tensor("yt", [2, 128, KC2, 2, HALF], fp8, kind="ExternalInput")
    # stationary slabs: [128, KC2, 2, ROWS]
    xbt = nc.dram_tensor("xbt", [128, KC2, 2, ROWS], fp8, kind="ExternalInput")
    ybt = nc.dram_tensor("ybt", [128, KC2, 2, ROWS], fp8, kind="ExternalInput")
    # column-bias rows -(|a_j|/sigma)^2/2 as hi/lo fp8 on one DR partition
    xaug = nc.dram_tensor("xaug", [1, 2, N], fp8, kind="ExternalInput")
    yaug = nc.dram_tensor("yaug", [1, 2, N], fp8, kind="ExternalInput")
    xbias = nc.dram_tensor("xbias", [128, MT], fp32, kind="ExternalInput")
    ybias = nc.dram_tensor("ybias", [128, MT], fp32, kind="ExternalInput")
    xbias2 = nc.dram_tensor("xbias2", [128, MT], fp32, kind="ExternalInput")
    ybias2 = nc.dram_tensor("ybias2", [128, MT], fp32, kind="ExternalInput")
    ones = nc.dram_tensor("ones", [1, 2, 128], fp8, kind="ExternalInput")

    rx_o = nc.dram_tensor("rx", [128, MT * 2], fp32, kind="ExternalOutput")
    ry_o = nc.dram_tensor("ry", [128, MT * 2], fp32, kind="ExternalOutput")
    # S partials: sxx/syy via ACT exp2 (+ optional DVE sst share), sxy DVE
    s_o = {
        "sxxa": nc.dram_tensor("sxxa", [128, MT * 2], fp32,
                               kind="ExternalOutput"),
        "syya": nc.dram_tensor("syya", [128, MT * 2], fp32,
                               kind="ExternalOutput"),
        "sxxd": nc.dram_tensor("sxxd", [128, MT], fp32,
                               kind="ExternalOutput"),
        "syyd": nc.dram_tensor("syyd", [128, MT], fp32,
                               kind="ExternalOutput"),
        "sxyd": nc.dram_tensor("sxyd", [128, MT], fp32,
                               kind="ExternalOutput"),
    }

    with tile.TileContext(nc) as tc:
        with (
            tc.tile_pool(name="res", bufs=1) as res,
            tc.tile_pool(name="kmat", bufs=1) as kpool,
            tc.tile_pool(name="psum", bufs=2, space="PSUM") as pp,
        ):
            # ---- persistent accumulators/scratch ----
            racc = {
                "x": res.tile([128, MT * 2], fp32, tag="rxacc", name="rxacc"),
                "y": res.tile([128, MT * 2], fp32, tag="ryacc", name="ryacc"),
            }
            s_acc = {}
            for nm, t in s_o.items():
                s_acc[nm] = res.tile(list(t.shape), fp32, tag=nm + "a",
                                     name=nm + "a")
            scr_d = res.tile([128, N], bf16, tag="scrd", name="scrd")
            scr_a = res.tile([128, HALF], bf16, tag="scra", name="scra")

            def body():
                if mode == "empty":
                    nc.scalar.memzero(racc["x"][:])
                    nc.scalar.memzero(racc["y"][:])
                    for nm in s_acc:
                        nc.scalar.memzero(s_acc[nm][:])
                    return
                # input tiles allocated per-iteration with bufs=2 so the next
                # rep's DMA overlaps this rep's compute (cross-rep pipelining)
                t_sb = {}
                bt_sb = {}
                aug_sb = {}
                bias_sb = {}
                bias2_sb = {}
                for mat in ("x", "y"):
                    for h in range(2):
                        t_sb[mat, h] = res.tile([128, KC2, 2, HALF], fp8,
                                                tag=f"{mat}t{h}",
                                                name=f"{mat}t{h}", bufs=2)
                    bt_sb[mat] = res.tile([128, KC2, 2, ROWS], fp8,
                                          tag=f"{mat}bt", name=f"{mat}bt",
                                          bufs=2)
                    aug_sb[mat] = res.tile([1, 2, N], fp8, tag=f"{mat}aug",
                                           name=f"{mat}aug", bufs=2)
                    bias_sb[mat] = res.tile([128, MT], fp32, tag=f"{mat}bias",
                                            name=f"{mat}bias", bufs=2)
                    bias2_sb[mat] = res.tile([128, MT], fp32,
                                             tag=f"{mat}bias2",
                                             name=f"{mat}bias2", bufs=2)
                ones_sb = res.tile([1, 2, 128], fp8, tag="ones",
                                   name="ones_sb", bufs=2)
                # small tensors on the ACT DGE queue, bulk on SP; X before Y
                nc.scalar.dma_start(ones_sb[:], ones[:])
                for mat, augdram, biasdram, bias2dram in (
                    ("x", xaug, xbias, xbias2),
                    ("y", yaug, ybias, ybias2),
                ):
                    nc.scalar.dma_start(aug_sb[mat][:], augdram[:])
                    nc.scalar.dma_start(bias_sb[mat][:], biasdram[:])
                    nc.scalar.dma_start(bias2_sb[mat][:], bias2dram[:])
                for mat, tdram, btdram in (("x", xt, xbt), ("y", yt, ybt)):
                    nc.sync.dma_start(bt_sb[mat][:], btdram[:])
                    for h in range(2):
                        nc.sync.dma_start(t_sb[mat, h][:], tdram[h])

                def krow(mat, m, ktile, sq_key=None):
                    """compute K[mat] rows [m*128, (m+1)*128) x [0, N)"""
                    for h in range(2):
                        g = pp.tile([128, HALF], fp32, tag="g", name="g")
                        stat = bt_sb[mat][:, :, :, m * 128:(m + 1) * 128]
                        for k in range(KC2):
                            for sub in range(HALF // MMN):
                                nc.tensor.matmul(
                                    g[:, sub * MMN:(sub + 1) * MMN],
                                    stat[:, k],
                                    t_sb[mat, h][:, k, :,
                                                 sub * MMN:(sub + 1) * MMN],
                                    start=(k == 0),
                                    stop=False,
                                    perf_mode=DR,
                                )
                        for sub in range(HALF // MMN):
                            base = h * HALF + sub * MMN
                            nc.tensor.matmul(
                                g[:, sub * MMN:(sub + 1) * MMN],
                                ones_sb[:],
                                aug_sb[mat][:, :, base:base + MMN],
                                start=False,
                                stop=True,
                                perf_mode=DR,
                            )
                        nc.scalar.activation(
                            ktile[:, h * HALF:(h + 1) * HALF],
                            g[:],
                            Exp,
                            bias=bias_sb[mat][:, m:m + 1],
                            scale=1.0,
                            accum_out=racc[mat][:, m * 2 + h:m * 2 + h + 1],
                        )
                        if sq_key is not None and (
                                exp2 == "all" or (exp2 == "h1" and h == 1)):
                            # exp(2*arg) = k^2: S self-sum on ScalarE, no
                            # product tile needed
                            nc.scalar.activation(
                                scr_a[:],
                                g[:],
                                Exp,
                                bias=bias2_sb[mat][:, m:m + 1],
                                scale=2.0,
                                accum_out=s_acc[sq_key][:, m * 2 + h:
                                                        m * 2 + h + 1],
                            )
                        elif sq_key is not None and exp2 == "part" and h == 1:
                            # light ScalarE share: square only the tail
                            # columns; VectorE ssts cover the rest
                            nc.scalar.activation(
                                scr_a[:, EXP2_START:],
                                g[:, EXP2_START:],
                                Exp,
                                bias=bias2_sb[mat][:, m:m + 1],
                                scale=2.0,
                                accum_out=s_acc[sq_key][:, m * 2 + h:
                                                        m * 2 + h + 1],
                            )

                def had_sum(a, b, dkey, split, m):
                    """accumulate sum(a[:, :split]*b[:, :split]) on DVE."""
                    nc.vector.scalar_tensor_tensor(
                        out=scr_d[:, :split], in0=a[:, :split], scalar=1.0,
                        in1=b[:, :split], op0=mult, op1=mult,
                        accum_out=s_acc[dkey][:, m:m + 1],
                    )

                if exp2 == "all":
                    nc.vector.memset(s_acc["sxxd"][:], 0.0)
                    nc.vector.memset(s_acc["syyd"][:], 0.0)
                else:
                    # h1/part modes: exp2 only writes odd cols of sxxa/syya
                    nc.vector.memset(s_acc["sxxa"][:], 0.0)
                    nc.vector.memset(s_acc["syya"][:], 0.0)
                if mode != "full":
                    # zero accumulators the stripped mode never writes
                    nc.scalar.memzero(racc["x"][:])
                    nc.scalar.memzero(racc["y"][:])
                    for nm in s_acc:
                        nc.scalar.memzero(s_acc[nm][:])
                if mode == "empty":
                    return
                if mode == "dma":
                    return
                # interleave x/y m-tiles so the product sums of tile m flow
                # right after y_m (keeps DVE/Pool fed, no tail pile-up)
                kbufs = 3 if deep else 2
                for m in range(MT):
                    kx = kpool.tile([128, N], bf16, tag="kx", name="kx",
                                    bufs=kbufs)
                    krow("x", m, kx, sq_key="sxxa")
                    if deep and mode == "full" and exp2 == "none":
                        had_sum(kx, kx, "sxxd", N, m)
                    ky = kpool.tile([128, N], bf16, tag="ky", name="ky",
                                    bufs=kbufs)
                    krow("y", m, ky, sq_key="syya")
                    if mode == "gram":
                        continue
                    if exp2 == "h1":
                        had_sum(kx, kx, "sxxd", HALF, m)
                        had_sum(ky, ky, "syyd", HALF, m)
                    elif exp2 == "part":
                        had_sum(kx, kx, "sxxd", HALF + EXP2_START, m)
                        had_sum(ky, ky, "syyd", HALF + EXP2_START, m)
                    elif exp2 == "none":
                        if not deep:
                            had_sum(kx, kx, "sxxd", N, m)
                        had_sum(ky, ky, "syyd", N, m)
                    had_sum(kx, ky, "sxyd", N, m)

            if reps == 1:
                body()
            else:
                with tc.For_i(0, reps, 1):
                    body()

            nc.sync.dma_start(rx_o[:], racc["x"][:])
            nc.sync.dma_start(ry_o[:], racc["y"][:])
            for nm in s_o:
                nc.sync.dma_start(s_o[nm][:], s_acc[nm][:])

    if not nc.is_finalized():
        nc.finalize()
    return nc


def _prep_matrix(A, inv_sigma):
    """Host-side: scale by 1/sigma, fp8 cast, aug hi/lo fp8 rows, row bias."""
    A8 = (A * inv_sigma).astype(FP8)
    d = (A8.astype(np.float64) ** 2).sum(axis=1)          # [N] |a/sigma|^2
    AT = np.ascontiguousarray(A8.T)                       # [D, N] fp8
    half = (-0.5 * d).astype(np.float32)                  # -(|a_j|/s)^2/2
    hi = half.astype(FP8)
    lo = (half - hi.astype(np.float32)).astype(FP8)
    aug = np.stack([hi, lo]).reshape(1, 2, N)             # [1, 2, N] fp8
    bias = half                                           # [N] fp32 exact
    return AT, aug, bias


def _t_layout(AT):
    """[768, N] fp8 -> [2, 128, KC2, 2, HALF] half-major DR layout."""
    a = AT.reshape(KC2, 2, 128, N).transpose(2, 0, 1, 3)      # [128,KC2,2,N]
    a = a.reshape(128, KC2, 2, 2, HALF).transpose(3, 0, 1, 2, 4)
    return np.ascontiguousarray(a)


def _bt_layout(AT_slice):
    """[768, ROWS] fp8 -> [128, KC2, 2, ROWS]."""
    return np.ascontiguousarray(
        AT_slice.reshape(KC2, 2, 128, ROWS).transpose(2, 0, 1, 3))


def _make_in_maps(X, Y, inv_sigma_sq):
    inv_sigma = float(np.sqrt(inv_sigma_sq))
    XT, xaug, xbias = _prep_matrix(X, inv_sigma)
    YT, yaug, ybias = _prep_matrix(Y, inv_sigma)
    ones = np.ones((1, 2, 128), dtype=FP8)
    xt_r = _t_layout(XT)
    yt_r = _t_layout(YT)

    in_maps = []
    for c in range(NCORES):
        sl = slice(c * ROWS, (c + 1) * ROWS)
        in_maps.append({
            "xt": xt_r,
            "yt": yt_r,
            "xbt": _bt_layout(XT[:, sl]),
            "ybt": _bt_layout(YT[:, sl]),
            "xaug": xaug,
            "yaug": yaug,
            "xbias": np.ascontiguousarray(xbias[sl].reshape(MT, 128).T),
            "ybias": np.ascontiguousarray(ybias[sl].reshape(MT, 128).T),
            "xbias2": np.ascontiguousarray(
                (2.0 * xbias[sl]).reshape(MT, 128).T),
            "ybias2": np.ascontiguousarray(
                (2.0 * ybias[sl]).reshape(MT, 128).T),
            "ones": ones,
        })
    return in_maps


def _combine(out):
    rx = np.empty(N, dtype=np.float64)
    ry = np.empty(N, dtype=np.float64)
    s_xx = s_yy = s_xy = 0.0
    for c in range(NCORES):
        r = out[c]
        rxc = r["rx"].astype(np.float64).reshape(128, MT, 2).sum(axis=2)
        ryc = r["ry"].astype(np.float64).reshape(128, MT, 2).sum(axis=2)
        rx[c * ROWS:(c + 1) * ROWS] = rxc.T.reshape(ROWS)
        ry[c * ROWS:(c + 1) * ROWS] = ryc.T.reshape(ROWS)
        s_xx += (r["sxxa"].astype(np.float64).sum()
                 + r["sxxd"].astype(np.float64).sum())
        s_yy += (r["syya"].astype(np.float64).sum()
                 + r["syyd"].astype(np.float64).sum())
        s_xy += r["sxyd"].astype(np.float64).sum()

    tx = rx.sum()
    ty = ry.sum()
    n = float(N)
    hsic_xy = s_xy - 2.0 / n * np.dot(rx, ry) + tx * ty / (n * n)
    hsic_xx = s_xx - 2.0 / n * np.dot(rx, rx) + tx * tx / (n * n)
    hsic_yy = s_yy - 2.0 / n * np.dot(ry, ry) + ty * ty / (n * n)
    return np.float32(hsic_xy / np.sqrt(hsic_xx * hsic_yy))


def kernel(X, Y, sigma, _reps=1, _mode="full", _exp2="none", _deep=True):
    from concourse.bass_utils import run_bass_kernel_spmd

    X = np.asarray(X, dtype=np.float32)
    Y = np.asarray(Y, dtype=np.float32)
    sig = float(np.asarray(sigma))
    inv_sigma_sq = 1.0 / (sig * sig)

    key = (inv_sigma_sq, _reps, _mode, _exp2, _deep)
    if key not in _cache:
        _cache[key] = _build(inv_sigma_sq, reps=_reps, mode=_mode,
                             exp2=_exp2, deep=_deep)
    nc = _cache[key]

    in_maps = _make_in_maps(X, Y, inv_sigma_sq)
    res = run_bass_kernel_spmd(nc, in_maps, list(range(NCORES)))
    global LAST_RESULTS
    LAST_RESULTS = res
    return _combine(res.results)

